# revision 19
# baseline (speedup 1.0000x reference)
"""Trainium2 Bass kernel for nn_BRC_62715112457019 (sparse_attention), v2.

Head-parallel across 8 cores (core c = head c, both samples). Pixel-major
phase A (per-pixel stats/masks live on partitions -> no broadcast DMAs, tiny
128-wide DVE ops), fp8 DoubleRow phase B (QK^T and AV at 2 fp8 MACs/cycle),
per-qchunk transposed epilogue (no row-broadcasts), overlapped channel-attn
path and output writeback.

Pixel blocking: block b in [0,18) covers pixels [128b, 128b+128). Pixel-major
tiles are [128, 18*K] with column group b. Channel-major tensors ([8|16, HW])
are produced/consumed via PE transposes per block.
"""
import sys
for _p in ('/opt/trn_rl_repo', '/opt/pypackages'):
    if _p not in sys.path:
        sys.path.insert(0, _p)
import numpy as np
import ml_dtypes
from contextlib import ExitStack

import concourse.bass as bass
import concourse.bacc as bacc
import concourse.tile as tile
from concourse import mybir

dt = mybir.dt
F32 = dt.float32
BF16 = dt.bfloat16
FP8 = dt.float8e4
AF = mybir.ActivationFunctionType
OP = mybir.AluOpType
DR = mybir.MatmulPerfMode.DoubleRow

HW = 2304
NB = 18                       # 128-pixel blocks
CHUNKS = [(0, 512), (512, 512), (1024, 512), (1536, 512), (2048, 256)]
GRP = 2                       # logit blocks per exp group
BF = ml_dtypes.bfloat16
F8 = ml_dtypes.float8_e4m3fn


def host_constants(w8, b8):
    eye = np.eye(128, dtype=np.float32)
    eyeb = np.eye(128, dtype=BF)
    selsum = np.zeros((128, 5 * 37), BF)
    for c in range(5):
        selsum[0:64, 37 * c + c] = 1.0
        selsum[64:128, 37 * c + 32 + c] = 1.0
    wb16 = np.zeros((16, 2), np.float32)
    wb16[0:8, 0] = w8
    wb16[8:16, 0] = w8
    wb16[0:8, 1] = b8
    wb16[8:16, 1] = b8
    ones16 = np.ones((16, 1), np.float32)
    rep = np.zeros((6, 48), np.float32)
    for m in range(3):          # fg, bb, b
        for s in range(2):
            rep[2 * m + s, 16 * m + 8 * s:16 * m + 8 * s + 8] = 1.0
    return {"eye": eye, "eyeb": eyeb, "selsum": selsum, "wb16": wb16,
            "ones16": ones16, "rep": rep}


def make_inmaps(F, P, norm_weight, norm_bias):
    F = np.asarray(F, np.float32).reshape(2, 64, HW)
    P = np.asarray(P, np.float32).reshape(2, 48, 48)
    w = np.asarray(norm_weight, np.float32)
    b = np.asarray(norm_bias, np.float32)
    maps = []
    for c in range(8):
        m = host_constants(w[8 * c:8 * c + 8], b[8 * c:8 * c + 8])
        order = np.r_[np.arange(8 * c, 8 * c + 8),
                      np.delete(np.arange(64), np.s_[8 * c:8 * c + 8])]
        for n in range(2):
            m[f"Fb{n}"] = np.ascontiguousarray(F[n][order].astype(BF))
            m[f"P{n}"] = np.ascontiguousarray(P[n])
        maps.append(m)
    return maps


def assemble(results):
    out = np.empty((2, 64, 48, 48), np.float32)
    for c in range(8):
        for n in range(2):
            out[n, 8 * c:8 * c + 8] = results[c][f"out{n}"].reshape(8, 48, 48)
    return out


def build_program(apply_wb):
    nc = bacc.Bacc("TRN2", target_bir_lowering=False, debug=False)
    ins = {}
    for n in range(2):
        ins[f"Fb{n}"] = nc.dram_tensor(f"Fb{n}", [64, HW], BF16, kind="ExternalInput").ap()
        ins[f"P{n}"] = nc.dram_tensor(f"P{n}", [48, 48], F32, kind="ExternalInput").ap()
    ins["eye"] = nc.dram_tensor("eye", [128, 128], F32, kind="ExternalInput").ap()
    ins["eyeb"] = nc.dram_tensor("eyeb", [128, 128], BF16, kind="ExternalInput").ap()
    ins["selsum"] = nc.dram_tensor("selsum", [128, 185], BF16, kind="ExternalInput").ap()
    ins["wb16"] = nc.dram_tensor("wb16", [16, 2], F32, kind="ExternalInput").ap()
    ins["ones16"] = nc.dram_tensor("ones16", [16, 1], F32, kind="ExternalInput").ap()
    ins["rep"] = nc.dram_tensor("rep", [6, 48], F32, kind="ExternalInput").ap()
    outs = [nc.dram_tensor(f"out{n}", [8, HW], F32, kind="ExternalOutput").ap() for n in range(2)]

    with tile.TileContext(nc) as tc:
        with ExitStack() as ctx:
            _body(ctx, tc, nc, ins, outs, apply_wb)
    nc.compile()
    return nc


# sobel slot indices (pairs of 50 cols: sample0|sample1, rows 0:48)
S_P50, S_PM, S_A1, S_TMP, S_B1, S_A1P, S_B1P, S_TCOL, S_GXT, S_GYT, S_M1, S_M2, \
    S_STT, S_BTM, S_BHW, S_FG, S_BG, S_BB = range(18)


def _body(ctx, tc, nc, ins, outs, apply_wb):
    pers = ctx.enter_context(tc.tile_pool(name="pers", bufs=1))
    sm = ctx.enter_context(tc.tile_pool(name="sm", bufs=1))

    # ---- persistent tiles ----
    eye = pers.tile([128, 128], F32, tag="eye")
    eyeb = pers.tile([128, 128], BF16, tag="eyeb")
    selsum = pers.tile([128, 185], BF16, tag="selsum")
    wb16 = pers.tile([16, 2], F32, tag="wb16")
    ones16 = pers.tile([16, 1], F32, tag="ones16")
    consts = pers.tile([128, 2], F32, tag="consts")     # col0 = eps
    F128 = pers.tile([128, HW], BF16, tag="F128")
    Fsq = pers.tile([128, HW], BF16, tag="Fsq")
    FnT = pers.tile([128, 288], F32, tag="FnT")         # 16b+8s+d
    qT = pers.tile([128, 288], BF16, tag="qT")
    NRM = pers.tile([128, 36], F32, tag="NRM")          # 2b+s
    RQB = pers.tile([128, 72], F32, tag="RQB")          # [0:36] sqrt, [36:72] recip
    MT = pers.tile([128, 296], F32, tag="MT")           # 74j+37t+32s+c
    MK3 = pers.tile([128, 18 * 48], F32, tag="MK3")     # 48b+{fg16,bb16,b16}
    mrows = pers.tile([6, HW], F32, tag="mrows")
    rep = pers.tile([6, 48], F32, tag="rep")
    qcm16 = pers.tile([16, HW], BF16, tag="qcm16")
    qcm1 = pers.tile([8, HW], BF16, tag="qcm1")
    trTav = [pers.tile([128, 288], FP8, tag=f"trTav{s}", name=f"trTav{s}") for s in range(2)]
    bfg24 = pers.tile([128, 864], F32, tag="bfg24")    # [b][s][fg8|bb8|Fn8]
    CM = [pers.tile([24, HW], BF16, tag=f"CM{s}", name=f"CM{s}") for s in range(2)]
    Sall = pers.tile([128, 2 * NB * 512], FP8, tag="Sall")
    w1 = pers.tile([128, 288], F32, tag="w1")
    spatT = pers.tile([128, 288], F32, tag="spatT")
    OUTT = pers.tile([128, 288], BF16, tag="OUTT")
    fin = [pers.tile([8, HW], F32, tag=f"fin{s}", name=f"fin{s}") for s in range(2)]
    rc = pers.tile([128, 8], F32, tag="rc")             # epilogue denominators
    sobm = pers.tile([48, 100 * 18], F32, tag="sobm")
    stm = pers.tile([37, 2560], F32, tag="stm")
    sq = pers.tile([128, 16], F32, tag="sq")
    # channel path smalls
    msk = pers.tile([16, 32], F32, tag="msk")
    r16f = pers.tile([16, 4], F32, tag="r16f")          # [0:2] sqrt, [2:4]=1/max(sqrt,..); col s
    rqd = pers.tile([8, 2], F32, tag="rqd")             # rq relocated to base 0
    A1 = pers.tile([8, 16], F32, tag="A1")
    A2 = pers.tile([8, 16], F32, tag="A2")
    expA = pers.tile([8, 16], F32, tag="expA")
    eden = pers.tile([8, 2], F32, tag="eden")
    rd8 = pers.tile([8, 2], F32, tag="rd8")
    rhs24T = pers.tile([8, 48], F32, tag="rhs24T")      # 24s col-block
    rhs24 = [pers.tile([24, 8], BF16, tag=f"rhs24_{s}", name=f"rhs24_{s}") for s in range(2)]
    WT = pers.tile([128, 16], F32, tag="WT") if apply_wb else None
    BT = pers.tile([128, 16], F32, tag="BT") if apply_wb else None

    def mu_col(b, s):
        return MT[:, 74 * (b % 4) + 32 * s + b // 4: 74 * (b % 4) + 32 * s + b // 4 + 1]

    def rs_col(b, s):
        o = 74 * (b % 4) + 37 + 32 * s + b // 4
        return MT[:, o:o + 1]

    mk3v = MK3[:].rearrange("p (b c) -> p b c", c=48)

    def mkv(m, s):
        return mk3v[:, :, 16 * m + 8 * s:16 * m + 8 * s + 8]

    fnv = FnT[:].rearrange("p (b c) -> p b c", c=16)
    w1v = w1[:].rearrange("p (b c) -> p b c", c=16)

    def fnsv(s):
        return fnv[:, :, 8 * s:8 * s + 8]

    def fnt(b, s):
        return FnT[:, 16 * b + 8 * s: 16 * b + 8 * s + 8]

    # =============== Phase A ===============
    with tc.tile_pool(name="psA", bufs=2, space="PSUM") as psA, \
         tc.tile_pool(name="psT", bufs=3, space="PSUM") as psT:

        nc.sync.dma_start(eye[:], ins["eye"])
        nc.sync.dma_start(eyeb[:], ins["eyeb"])
        nc.sync.dma_start(selsum[:], ins["selsum"])
        nc.sync.dma_start(wb16[:], ins["wb16"])
        nc.sync.dma_start(ones16[:], ins["ones16"])
        nc.sync.dma_start(rep[:], ins["rep"])
        nc.vector.memset(consts[:, 0:1], 1e-5)
        for s in range(2):
            nc.vector.memset(trTav[s][:], 0.0)
        for off, w in CHUNKS:
            nc.sync.dma_start(F128[0:64, off:off + w], ins["Fb0"][:, off:off + w])
            nc.sync.dma_start(F128[64:128, off:off + w], ins["Fb1"][:, off:off + w])

        # ---- masks (sobel in image space, batched samples in free dim) ----
        sv = sobm[:].rearrange("p (i s c) -> p i s c", s=2, c=50)

        def slot(i, r=(1, 49)):
            return sv[:, i, :, r[0]:r[1]]

        nc.gpsimd.memset(sobm[:, 0:200], 0.0)
        nc.sync.dma_start(slot(S_P50)[:, 0, :], ins["P0"])
        nc.sync.dma_start(slot(S_P50)[:, 1, :], ins["P1"])
        nc.scalar.activation(slot(S_PM), slot(S_P50), AF.Sigmoid)
        # exp table warmup (avoid mid-phase-B table load)
        nc.scalar.activation(sq[0:1, 0:1], consts[0:1, 0:1], AF.Exp)
        Pm0 = sv[:, S_PM]
        nc.vector.tensor_tensor(slot(S_A1), Pm0[:, :, 0:48], Pm0[:, :, 2:50], OP.subtract)
        nc.vector.tensor_tensor(slot(S_TMP), Pm0[:, :, 0:48], Pm0[:, :, 2:50], OP.add)
        nc.vector.scalar_tensor_tensor(slot(S_B1), Pm0[:, :, 1:49], 2.0, slot(S_TMP),
                                       OP.mult, OP.add)
        nc.gpsimd.memset(sobm[:, 100 * S_A1P:100 * S_A1P + 200], 0.0)
        for s in range(2):
            pt1 = psT.tile([128, 512], F32, tag="psT", name=f"pt1_{s}")
            nc.tensor.transpose(pt1[0:48, 0:48], slot(S_A1)[:, s, :], eye[0:48, 0:48])
            nc.vector.tensor_copy(slot(S_A1P)[:, s, :], pt1[0:48, 0:48])
            pt2 = psT.tile([128, 512], F32, tag="psT", name=f"pt2_{s}")
            nc.tensor.transpose(pt2[0:48, 0:48], slot(S_B1)[:, s, :], eye[0:48, 0:48])
            nc.vector.tensor_copy(slot(S_B1P)[:, s, :], pt2[0:48, 0:48])
        A1p = sv[:, S_A1P]
        B1p = sv[:, S_B1P]
        nc.vector.tensor_tensor(slot(S_TCOL), A1p[:, :, 0:48], A1p[:, :, 2:50], OP.add)
        nc.vector.scalar_tensor_tensor(slot(S_GXT), A1p[:, :, 1:49], 2.0, slot(S_TCOL),
                                       OP.mult, OP.add)
        nc.vector.tensor_tensor(slot(S_GYT), B1p[:, :, 0:48], B1p[:, :, 2:50], OP.subtract)
        nc.vector.tensor_tensor(slot(S_M1), slot(S_GXT), slot(S_GXT), OP.mult)
        nc.vector.tensor_tensor(slot(S_M2), slot(S_GYT), slot(S_GYT), OP.mult)
        nc.vector.tensor_tensor(slot(S_STT), slot(S_M1), slot(S_M2), OP.add)
        nc.vector.tensor_scalar(slot(S_BTM), slot(S_STT), 0.0, None, OP.is_gt)
        for s in range(2):
            pt3 = psT.tile([128, 512], F32, tag="psT", name=f"pt3_{s}")
            nc.tensor.transpose(pt3[0:48, 0:48], slot(S_BTM)[:, s, :], eye[0:48, 0:48])
            nc.vector.tensor_copy(slot(S_BHW)[:, s, :], pt3[0:48, 0:48])
        nc.vector.tensor_scalar(slot(S_FG), slot(S_P50), 0.0, None, OP.is_gt)
        nc.vector.tensor_scalar(slot(S_BG), slot(S_P50), 0.0, None, OP.is_lt)
        nc.vector.scalar_tensor_tensor(slot(S_BB), slot(S_BG), 1.0, slot(S_BHW),
                                       OP.mult, OP.max)
        # flatten masks to rows, then per-block transposes -> MKT
        for m, si in ((0, S_FG), (2, S_BB), (4, S_BHW)):
            for s in range(2):
                nc.sync.dma_start(mrows[m + s:m + s + 1, :], slot(si)[:, s, :])
        for b in range(NB):
            pm = psT.tile([128, 512], F32, tag="psT", name=f"pm{b}")
            nc.tensor.matmul(pm[0:128, 0:48], mrows[:, 128 * b:128 * b + 128],
                             rep[:], start=True, stop=True)
            nc.vector.tensor_copy(MK3[:, 48 * b:48 * b + 48], pm[0:128, 0:48])

        # ---- LayerNorm stats (channel-major PE reduction) ----
        for off, w in CHUNKS:
            nc.vector.tensor_tensor(Fsq[:, off:off + w], F128[:, off:off + w],
                                    F128[:, off:off + w], OP.mult)
        psumsA = psA.tile([128, 512], F32, tag="psA", name="psumsA")
        psumsB = psA.tile([128, 512], F32, tag="psA", name="psumsB")
        for c, (off, w) in enumerate(CHUNKS):
            nc.tensor.matmul(psumsA[0:37, 0:w], selsum[:, 37 * c:37 * c + 37],
                             F128[:, off:off + w], start=(c == 0), stop=(c == 4))
            nc.tensor.matmul(psumsB[0:37, 0:w], selsum[:, 37 * c:37 * c + 37],
                             Fsq[:, off:off + w], start=(c == 0), stop=(c == 4))
        s2 = stm[:, 0:512]
        varT = stm[:, 512:1024]
        sd = stm[:, 1024:1536]
        rstd = stm[:, 1536:2048]
        mu = stm[:, 2048:2560]
        nc.scalar.activation(s2, psumsA[0:37, :], AF.Square, scale=0.125)
        nc.vector.scalar_tensor_tensor(varT, psumsB[0:37, :], 1.0, s2, OP.mult, OP.subtract)
        nc.scalar.activation(sd, varT, AF.Sqrt, bias=consts[0:37, 0:1], scale=1.0 / 64.0)
        nc.vector.reciprocal(rstd, sd)
        nc.vector.tensor_scalar(mu, psumsA[0:37, :], 1.0 / 64.0, None, OP.mult)
        # stats -> pixel-major MT via transposes
        for j in range(4):
            for t, src in ((0, mu), (1, rstd)):
                pst = psT.tile([128, 512], F32, tag="psT", name=f"pst{j}_{t}")
                nc.tensor.transpose(pst[0:128, 0:37], src[:, 128 * j:128 * j + 128],
                                    eye[0:37, 0:37])
                nc.vector.tensor_copy(MT[:, 74 * j + 37 * t:74 * j + 37 * t + 37],
                                      pst[0:128, 0:37])

        if apply_wb:
            # broadcast per-channel w/b across partitions via ones-matmul
            pw = psT.tile([128, 512], F32, tag="psT", name="pw")
            nc.tensor.transpose(pw[0:2, 0:16], wb16[:], eye[0:16, 0:16])
            wbrow = sm.tile([2, 16], F32, tag="wbrow")
            nc.vector.tensor_copy(wbrow[:], pw[0:2, 0:16])
            onesr = sm.tile([1, 128], F32, tag="onesr")
            nc.vector.memset(onesr[:], 1.0)
            pw2 = psT.tile([128, 512], F32, tag="psT", name="pw2")
            nc.tensor.matmul(pw2[0:128, 0:16], onesr[:], wbrow[0:1, :], start=True, stop=True)
            nc.vector.tensor_copy(WT[:], pw2[0:128, 0:16])
            pw3 = psT.tile([128, 512], F32, tag="psT", name="pw3")
            nc.tensor.matmul(pw3[0:128, 0:16], onesr[:], wbrow[1:2, :], start=True, stop=True)
            nc.vector.tensor_copy(BT[:], pw3[0:128, 0:16])

        # ---- per-block: Fn_T, norms, q_T ----
        for b in range(NB):
            pF = psT.tile([128, 512], F32, tag="psT", name=f"pF{b}")
            pFb = pF[0:128, 0:8].bitcast(BF16)
            nc.tensor.transpose(pFb[:, 0:8], F128[0:8, 128 * b:128 * b + 128], eyeb[0:8, 0:8])
            nc.tensor.transpose(pFb[:, 8:16], F128[64:72, 128 * b:128 * b + 128],
                                eyeb[64:72, 64:72])
            for s in range(2):
                nc.vector.tensor_scalar(fnt(b, s), pFb[:, 8 * s:8 * s + 8],
                                        mu_col(b, s), rs_col(b, s), OP.subtract, OP.mult)
            if apply_wb:
                nc.vector.tensor_tensor(FnT[:, 16 * b:16 * b + 16],
                                        FnT[:, 16 * b:16 * b + 16], WT[:], OP.mult)
                nc.vector.tensor_tensor(FnT[:, 16 * b:16 * b + 16],
                                        FnT[:, 16 * b:16 * b + 16], BT[:], OP.add)
            for s in range(2):
                nc.vector.scalar_tensor_tensor(sq[:, 8 * s:8 * s + 8], fnt(b, s), 1.0,
                                               fnt(b, s), OP.mult, OP.mult,
                                               accum_out=NRM[:, 2 * b + s:2 * b + s + 1])
        nc.scalar.activation(RQB[:, 0:36], NRM[:], AF.Sqrt)
        nc.vector.tensor_scalar(RQB[:, 0:36], RQB[:, 0:36], 1e-12, None, OP.max)
        nc.vector.reciprocal(RQB[:, 36:72], RQB[:, 0:36])
        qtv = qT[:].rearrange("p (b s c) -> p b s c", s=2, c=8)
        fnv4 = FnT[:].rearrange("p (b s c) -> p b s c", s=2, c=8)
        rqbc = RQB[:, 36:72].rearrange("p (b s) -> p b s", s=2).unsqueeze(3) \
            .broadcast_to([128, 18, 2, 8])
        nc.vector.tensor_tensor(qtv[:], fnv4[:], rqbc, OP.mult)
        for b in range(NB):
            pQ = psT.tile([128, 512], F32, tag="psT", name=f"pQ{b}")
            pQb = pQ[0:16, 0:64].bitcast(BF16)
            nc.tensor.transpose(pQb, qT[:, 16 * b:16 * b + 16], eyeb[:, :])
            nc.scalar.activation(qcm16[:, 128 * b:128 * b + 128], pQb, AF.Copy)
        nc.sync.dma_start(qcm1[:], qcm16[8:16, :])

        # ---- masked features: batched full-tile ops over 3D views ----
        b24 = bfg24[:].rearrange("p (b s c) -> p b s c", s=2, c=24)
        tvv = [trTav[s][:].rearrange("p (b c) -> p b c", c=16) for s in range(2)]
        for s in range(2):
            nc.vector.tensor_tensor(tvv[s][:, :, 0:8], fnsv(s), mkv(0, s), OP.mult)
            nc.vector.tensor_copy(tvv[s][:, :, 8:9], mk3v[:, :, 8 * s:8 * s + 1])
            nc.vector.tensor_tensor(b24[:, :, s, 0:8], fnsv(s), mkv(0, s), OP.mult)
            nc.vector.tensor_tensor(b24[:, :, s, 8:16], fnsv(s), mkv(1, s), OP.mult)
            nc.vector.tensor_copy(b24[:, :, s, 16:24], fnsv(s))
        for b in range(NB):
            for s in range(2):
                pC = psT.tile([128, 512], F32, tag="psT", name=f"pC{b}_{s}")
                nc.tensor.transpose(pC[0:24, 0:128],
                                    bfg24[:, 48 * b + 24 * s:48 * b + 24 * s + 24],
                                    eye[:, :])
                nc.vector.tensor_copy(CM[s][:, 128 * b:128 * b + 128],
                                      pC[0:24, 0:128])
        # w1 = Fn + b*(q - Fn)   (final out = w1 + Fn + b*spat + Fch + qc)
        nc.vector.tensor_tensor(w1[:], qT[:], FnT[:], OP.subtract)
        for s in range(2):
            nc.vector.tensor_tensor(w1v[:, :, 8 * s:8 * s + 8], w1v[:, :, 8 * s:8 * s + 8],
                                    mkv(2, s), OP.mult)
        nc.vector.tensor_tensor(w1[:], w1[:], FnT[:], OP.add)

    # =============== Phase B + channel path + finals ===============
    Sv = Sall[:].rearrange("p (t b x) -> p t b x", t=2, b=NB)
    tv = [trTav[s][:].rearrange("p (pb i c) -> p pb i c", i=2, c=16) for s in range(2)]

    with tc.tile_pool(name="psL", bufs=2, space="PSUM") as psL, \
         tc.tile_pool(name="psO", bufs=2, space="PSUM") as psO, \
         tc.tile_pool(name="psM", bufs=1, space="PSUM") as psMp, \
         tc.tile_pool(name="sS", bufs=2) as sS:
        psM = psMp.tile([128, 512], F32, tag="psM")
        kslot = [0]

        def phase_b(s, jc, joff, jw, gq):
            qsrc = qcm16[0:8, :] if s == 0 else qcm1[:]
            if True:
                Sb = Sv[:, gq % 2]
                psOt = psO.tile([128, 512], F32, tag="psO", name=f"psO{s}_{jc}")

                if pending and (s + jc) > 0:
                    finals_chunk(*pending.pop(0))

                def logits(g):
                    Lg = psL.tile([128, GRP * 512], F32, tag="L", name=f"L{s}_{jc}_{g}")
                    Lv = Lg[:].rearrange("p (i x) -> p i x", i=GRP)
                    for i in range(GRP):
                        b = GRP * g + i
                        nc.tensor.matmul(Lv[:, i, 0:jw], qsrc[:, 128 * b:128 * b + 128],
                                         qsrc[:, joff:joff + jw],
                                         start=True, stop=True)
                    nc.scalar.activation(Sb[:, GRP * g:GRP * g + GRP, 0:jw],
                                         Lv[:, :, 0:jw], AF.Exp)

                def av(pb):
                    nc.tensor.matmul(psOt[0:16, 0:jw], tv[s][:, pb], Sb[:, 2 * pb:2 * pb + 2, 0:jw],
                                     start=(pb == 0), stop=(pb == 8), perf_mode=DR)

                logits(0)
                for g in range(1, 9):
                    logits(g)
                    av(g - 1)
                av(8)
                # transposed epilogue: spat = num/den, pixel-major
                avs = sS.tile([9, 512], F32, tag="avs", name=f"avs{s}_{jc}")
                nc.vector.tensor_copy(avs[:, 0:jw], psOt[0:9, 0:jw])
                for j in range(jw // 128):
                    b = 4 * jc + j
                    k = kslot[0] % 8
                    kslot[0] += 1
                    pslot = psM[0:128, 9 * k:9 * k + 9]
                    nc.tensor.transpose(pslot, avs[:, 128 * j:128 * j + 128], eye[0:9, 0:9])
                    nc.vector.reciprocal(rc[:, k:k + 1], pslot[:, 8:9])
                    nc.vector.tensor_scalar(spatT[:, 16 * b + 8 * s:16 * b + 8 * s + 8],
                                            pslot[:, 0:8], rc[:, k:k + 1], None, OP.mult)
                pending.append((s, jc, joff, jw))

        def channel_path():
            # per-sample Gram accumulation: psum16 = [fg|bb].T @ [fg|bb] over pixels
            ps16 = [psM[0:16, 72 + 16 * s:72 + 16 * s + 16] for s in range(2)]
            for s in range(2):
                for b in range(NB):
                    ap = bfg24[:, 48 * b + 24 * s:48 * b + 24 * s + 16]
                    nc.tensor.matmul(ps16[s], ap, ap,
                                     start=(b == 0), stop=(b == NB - 1))
            for s in range(2):
                nc.vector.tensor_tensor(msk[:, 16 * s:16 * s + 16], ps16[s],
                                        eye[0:16, 0:16], OP.mult)
                pd = psM[0:16, 104 + 2 * s:104 + 2 * s + 1]
                nc.tensor.matmul(pd, msk[:, 16 * s:16 * s + 16], ones16[:],
                                 start=True, stop=True)
                nc.scalar.activation(r16f[:, s:s + 1], pd, AF.Sqrt)
            nc.vector.tensor_scalar(r16f[:, 0:2], r16f[:, 0:2], 1e-12, None, OP.max)
            nc.vector.reciprocal(r16f[:, 2:4], r16f[:, 0:2])
            # rq (rows 8:16 of r16f) relocated to base 0
            nc.sync.dma_start(rqd[:], r16f[8:16, 2:4])
            for s in range(2):
                # Gram is symmetric: G^T[k, q] = Gram[0:8, 8:16] (fg rows, bb cols)
                nc.vector.tensor_scalar(A1[:, 8 * s:8 * s + 8],
                                        ps16[s][0:8, 8:16],
                                        r16f[0:8, 2 + s:3 + s], None, OP.mult)
                pA = psM[0:8, 140 + 8 * s:140 + 8 * s + 8]
                nc.tensor.transpose(pA, A1[:, 8 * s:8 * s + 8], eye[0:8, 0:8])
                nc.vector.tensor_scalar(A2[:, 8 * s:8 * s + 8], pA, rqd[:, s:s + 1],
                                        None, OP.mult)
                nc.scalar.activation(expA[:, 8 * s:8 * s + 8], A2[:, 8 * s:8 * s + 8],
                                     AF.Exp, accum_out=eden[:, s:s + 1])
                nc.vector.reciprocal(rd8[:, s:s + 1], eden[:, s:s + 1])
                # rhs24T[q, :] = [attn_c[q, k], diag(rq)[q, k], I8[q, d]] -> transpose
                base = 24 * s
                nc.vector.tensor_scalar(rhs24T[:, base:base + 8], expA[:, 8 * s:8 * s + 8],
                                        rd8[:, s:s + 1], None, OP.mult)
                nc.vector.tensor_scalar(rhs24T[:, base + 8:base + 16], eye[0:8, 0:8],
                                        rqd[:, s:s + 1], None, OP.mult)
                nc.vector.tensor_copy(rhs24T[:, base + 16:base + 24], eye[0:8, 0:8])
                pR = psM[0:24, 156 + 8 * s:156 + 8 * s + 8]
                nc.tensor.transpose(pR, rhs24T[:, base:base + 24], eye[0:8, 0:8])
                nc.vector.tensor_copy(rhs24[s][:], pR)

        spv = spatT[:].rearrange("p (b c) -> p b c", c=16)
        ov = OUTT[:].rearrange("p (b c) -> p b c", c=16)

        def finals_chunk(s, jc, joff, jw):
            nb = jw // 128
            bs = slice(4 * jc, 4 * jc + nb)
            nc.vector.tensor_tensor(ov[:, bs, 8 * s:8 * s + 8], spv[:, bs, 8 * s:8 * s + 8],
                                    mkv(2, s)[:, bs, :], OP.mult)
            nc.vector.tensor_tensor(ov[:, bs, 8 * s:8 * s + 8], ov[:, bs, 8 * s:8 * s + 8],
                                    w1v[:, bs, 8 * s:8 * s + 8], OP.add)
            for b in range(4 * jc, 4 * jc + nb):
                fslot = psM[0:128, 172 + 8 * (b % 2):172 + 8 * (b % 2) + 8]
                nc.tensor.matmul(fslot, CM[s][:, 128 * b:128 * b + 128], rhs24[s][:],
                                 start=True, stop=True)
                o = OUTT[:, 16 * b + 8 * s:16 * b + 8 * s + 8]
                nc.vector.tensor_tensor(o, o, fslot, OP.add)
                tslot = psM[0:8, 192 + 64 * (b % 2):192 + 64 * (b % 2) + 64].bitcast(BF16)
                nc.tensor.transpose(tslot, o, eyeb[:, :])
                nc.vector.tensor_copy(fin[s][:, 128 * b:128 * b + 128], tslot)
            nc.sync.dma_start(outs[s][:, joff:joff + jw], fin[s][:, joff:joff + jw])

        channel_path()
        pending = []

        def flush_pending():
            while pending:
                finals_chunk(*pending.pop(0))

        gq = 0
        for jc, (joff, jw) in enumerate(CHUNKS):
            for s in range(2):
                phase_b(s, jc, joff, jw, gq)
                gq += 1
        flush_pending()


_PROGRAMS = {}


def _program(apply_wb=False):
    if apply_wb not in _PROGRAMS:
        _PROGRAMS[apply_wb] = build_program(apply_wb)
    return _PROGRAMS[apply_wb]


def kernel(F, P, norm_weight, norm_bias):
    from concourse.bass_utils import run_bass_kernel_spmd
    w = np.asarray(norm_weight, np.float32)
    b = np.asarray(norm_bias, np.float32)
    apply_wb = not (np.all(w == 1.0) and np.all(b == 0.0))
    nc = _program(apply_wb)
    maps = make_inmaps(F, P, norm_weight, norm_bias)
    res = run_bass_kernel_spmd(nc, maps, core_ids=list(range(8)), trace=False)
    return assemble(res.results)
